# revision 10
# baseline (speedup 1.0000x reference)
"""Trainium2 Bass kernel for CoAttention_TextImage.

Math: in both co-attention stages the query-side score is constant along
the softmax axis, so it cancels inside softmax:
  visual_att[b,s,:]  = softmax_r(si[b,:])   (independent of s)
  textual_att[b,s,:] = softmax_t(sk[b,:])   (independent of s)
Therefore each output is one per-batch vector broadcast over S:
  att_img[b,s,:]  = softmax(tanh(img[b]@W_i1)@w_a1[H:])  @ img[b]
  att_text[b,s,:] = softmax(tanh(text[b]@W_t2)@w_a2[H:]) @ text[b]
(W_t1, b_t1, W_i2, b_i2, w_a1[:H], w_a2[:H], b_a1, b_a2 cancel exactly.)

Sharding: 8 cores, one uniform SPMD program. Cores 0-3 run the text side
(2 batches each, W=W_t2), cores 4-7 the img side (2 batches each, W=W_i1,
rows zero-padded 49->128 with an additive -1e30 softmax mask). Each core
loads one (768,768) weight + its activations; no cross-core comm.

Per-core device program ("seg" = one batch element, 2 segs/core):
  XT = transpose(X_seg)                  (PE transposes, 6x 128x128)
  Y  = X_seg @ W                         (PE fp32r, XT stationary, W moving)
  T  = tanh(Y)                           (ACT)
  s  = reduce_free(T * wa_bcast)         (DVE mult + reduce, per column half)
  e  = exp(s + mask)                     (ACT, mask as bias)
  u  = e.T @ [X_seg | 1]                 (PE fp32r; u[:768] unnormalized, u[768]=Z)
  v  = u[:768] / Z                       (DVE reciprocal + tensor_scalar)

Perf notes:
- The X/W datapath is float32r (PE single-pass: 1 cycle/col vs 4 for
  fp32; measured HW matmul rel err 1.5e-4, softmax-damped in the output).
- W is DMA'd in 12 column-half chunks, half 0 first, and the matmul loop
  is half-major, so tanh + score-reduce for half 0 overlap half 1's DMA.
- wa_bcast is built on-device (tiny row DMA + PE ones-broadcast, exact
  fp32) instead of a 0.4MB broadcast DMA.
- All DMAs issue from the SP sequencer (each dma_start occupies the
  issuing engine's SEQ ~650ns; spreading to ACT/Pool lengthens their
  drains and slows the tail).
Host broadcasts v over S and assembles the full outputs.
"""

import sys

if "/opt/trn_rl_repo" not in sys.path:
    sys.path.insert(0, "/opt/trn_rl_repo")

import numpy as np

import concourse.bass as bass
import concourse.bacc as bacc
import concourse.tile as tile
from concourse import mybir
from concourse.bass_utils import run_bass_kernel_spmd
from concourse.masks import make_identity

F32 = mybir.dt.float32
F32R = mybir.dt.float32r
B, S, R, H = 8, 128, 49, 768
KT = H // 128  # 6 contraction tiles
SEGS = 2       # batches per core
NH = 2         # column halves of 384
NCORES = 8
ALU = mybir.AluOpType
AF = mybir.ActivationFunctionType

_cache = {}


def build_program():
    if "nc" in _cache:
        return _cache["nc"]

    nc = bacc.Bacc("TRN2", target_bir_lowering=False, debug=False)

    W = nc.dram_tensor("W", [H, H], F32R, kind="ExternalInput")
    X = nc.dram_tensor("X", [SEGS, 128, H], F32R, kind="ExternalInput")
    MASK = nc.dram_tensor("MASK", [SEGS, 128], F32, kind="ExternalInput")
    WA = nc.dram_tensor("WA", [H], F32, kind="ExternalInput")
    V = nc.dram_tensor("V", [SEGS, H], F32, kind="ExternalOutput")

    with tile.TileContext(nc) as tc:
        with (
            tc.tile_pool(name="const", bufs=1) as const,
            tc.tile_pool(name="data", bufs=1) as data,
            tc.tile_pool(name="scratch", bufs=2) as scratch,
            tc.tile_pool(name="xtp", bufs=2, space="PSUM") as xtp,
            tc.tile_pool(name="ypsum", bufs=1, space="PSUM") as ypsum,
            tc.tile_pool(name="upsum", bufs=1, space="PSUM") as upsum,
        ):
            # identity in fp32 (memset/affine_select have no fp32r flavor),
            # then a typed copy so the fp32r transposes see an fp32r producer
            ident = const.tile([128, 128], F32)
            make_identity(nc, ident)
            identr = const.tile([128, 128], F32R)
            nc.vector.tensor_copy(out=identr[:], in_=ident[:])

            # wa broadcast to 128 partitions: tiny row DMA + PE ones-broadcast
            wa_row = const.tile([1, H], F32)
            nc.sync.dma_start(out=wa_row[:], in_=WA[:])
            ones_col = const.tile([1, 128], F32)
            nc.vector.memset(ones_col[:], 1.0)
            wab = const.tile([128, H], F32)
            for nh in range(NH):
                wp = xtp.tile([128, 384], F32, name=f"wp{nh}", tag="pt")
                nc.tensor.matmul(
                    wp[:], lhsT=ones_col[:],
                    rhs=wa_row[:, nh * 384 : (nh + 1) * 384],
                    start=True, stop=True,
                )
                nc.vector.tensor_copy(out=wab[:, nh * 384 : (nh + 1) * 384], in_=wp[:])

            # additive softmax mask, transposed to [128 rows, seg]
            maskT = const.tile([128, SEGS], F32)
            nc.sync.dma_start(out=maskT[:], in_=MASK[:].rearrange("s p -> p s"))

            # X in natural layout [row, seg, h], with ones-columns at h=H,H+1
            # (two so the second u-matmul chunk keeps an even fp32r width)
            ones_part = const.tile([128, 1], F32)
            nc.vector.memset(ones_part[:], 1.0)
            xsb = data.tile([128, SEGS, H + 2], F32R)
            nc.sync.dma_start(out=xsb[:, :, 0:H], in_=X[:].rearrange("s p h -> p s h"))
            for s in range(SEGS):
                nc.vector.tensor_copy(out=xsb[:, s, H : H + 1], in_=ones_part[:])
                nc.vector.tensor_copy(out=xsb[:, s, H + 1 : H + 2], in_=ones_part[:])

            # W tiles [k, kt, n]: 12 chunks, column-half 0 first
            wsb = data.tile([128, KT, H], F32R)
            Wr = W[:].rearrange("(t p) n -> t p n", p=128)
            for nh in range(NH):
                for kt in range(KT):
                    nc.sync.dma_start(
                        out=wsb[:, kt, nh * 384 : (nh + 1) * 384],
                        in_=Wr[kt, :, nh * 384 : (nh + 1) * 384],
                    )

            # transpose X -> XT (stationary operands for stage 1)
            xtsb = data.tile([128, SEGS, KT, 128], F32R)
            for s in range(SEGS):
                for kt in range(KT):
                    pt = xtp.tile([128, 128], F32R, tag="pt")
                    nc.tensor.transpose(
                        pt[:], xsb[:, s, kt * 128 : (kt + 1) * 128], identr[:]
                    )
                    nc.scalar.copy(out=xtsb[:, s, kt, :], in_=pt[:])

            # stage 1 (half-major): Y[s][:, half] = X_seg @ W[:, half];
            # score partials for half 0 run while half 1 streams in
            y = [
                [
                    ypsum.tile([128, 384], F32, name=f"y{s}{nh}", tag=f"y{s}{nh}")
                    for nh in range(NH)
                ]
                for s in range(SEGS)
            ]
            t1 = data.tile([128, SEGS, H], F32)
            sschalf = data.tile([128, SEGS, NH], F32)
            ssc = data.tile([128, SEGS], F32)
            esc = data.tile([128, SEGS], F32R)
            vsb = data.tile([1, SEGS, H], F32)
            for nh in range(NH):
                for kt in range(KT):
                    for s in range(SEGS):
                        nc.tensor.matmul(
                            y[s][nh][:],
                            lhsT=xtsb[:, s, kt, :],
                            rhs=wsb[:, kt, nh * 384 : (nh + 1) * 384],
                            start=(kt == 0),
                            stop=(kt == KT - 1),
                        )
                for s in range(SEGS):
                    nc.scalar.activation(
                        out=t1[:, s, nh * 384 : (nh + 1) * 384],
                        in_=y[s][nh][:],
                        func=AF.Tanh,
                    )
                    prod = scratch.tile([128, 384], F32, tag="prod")
                    nc.vector.tensor_tensor(
                        out=prod[:],
                        in0=t1[:, s, nh * 384 : (nh + 1) * 384],
                        in1=wab[:, nh * 384 : (nh + 1) * 384],
                        op=ALU.mult,
                    )
                    nc.vector.tensor_reduce(
                        out=sschalf[:, s, nh : nh + 1], in_=prod[:],
                        axis=mybir.AxisListType.X, op=ALU.add,
                    )
            for s in range(SEGS):
                nc.vector.tensor_reduce(
                    out=ssc[:, s : s + 1], in_=sschalf[:, s, :],
                    axis=mybir.AxisListType.X, op=ALU.add,
                )
                nc.scalar.activation(
                    out=esc[:, s : s + 1],
                    in_=ssc[:, s : s + 1],
                    func=AF.Exp,
                    bias=maskT[:, s : s + 1],
                )
                # u = e.T @ [X | 1]  -> u[0:768] unnormalized, u[768] = Z
                u0 = upsum.tile([1, 512], F32, tag="u0")
                u1 = upsum.tile([1, 258], F32, tag="u1")
                nc.tensor.matmul(
                    u0[:], lhsT=esc[:, s : s + 1], rhs=xsb[:, s, 0:512],
                    start=True, stop=True,
                )
                nc.tensor.matmul(
                    u1[:], lhsT=esc[:, s : s + 1], rhs=xsb[:, s, 512 : H + 2],
                    start=True, stop=True,
                )
                zr = scratch.tile([1, 1], F32, tag="zr")
                nc.vector.reciprocal(out=zr[:], in_=u1[0:1, 256:257])
                nc.vector.tensor_scalar_mul(vsb[:, s, 0:512], u0[0:1, :], zr[:])
                nc.vector.tensor_scalar_mul(vsb[:, s, 512:H], u1[0:1, 0:256], zr[:])
            nc.sync.dma_start(out=V[:], in_=vsb[0:1, :, :])

    nc.compile()
    _cache["nc"] = nc
    return nc


def make_in_maps(text, img, W_t2, W_i1, wa2, wa1):
    """Per-core input dicts. Cores 0-3: text side; cores 4-7: img side."""
    in_maps = []
    mask_t = np.zeros((SEGS, 128), np.float32)
    for c in range(4):
        in_maps.append(
            {
                "W": W_t2,
                "X": np.ascontiguousarray(text[2 * c : 2 * c + 2]),
                "MASK": mask_t,
                "WA": wa2,
            }
        )
    mask_i = np.zeros((SEGS, 128), np.float32)
    mask_i[:, R:] = -1e30
    for c in range(4):
        Xp = np.zeros((SEGS, 128, H), np.float32)
        Xp[:, :R, :] = img[2 * c : 2 * c + 2]
        in_maps.append({"W": W_i1, "X": Xp, "MASK": mask_i, "WA": wa1})
    return in_maps


def kernel(**inputs):
    text = np.ascontiguousarray(np.asarray(inputs["text_features"], np.float32))
    img = np.ascontiguousarray(np.asarray(inputs["img_features"], np.float32))
    W_t2 = np.ascontiguousarray(np.asarray(inputs["W_t2"], np.float32))
    W_i1 = np.ascontiguousarray(np.asarray(inputs["W_i1"], np.float32))
    wa2 = np.ascontiguousarray(np.asarray(inputs["w_a2"], np.float32)[H:])
    wa1 = np.ascontiguousarray(np.asarray(inputs["w_a1"], np.float32)[H:])

    nc = build_program()
    in_maps = make_in_maps(text, img, W_t2, W_i1, wa2, wa1)
    res = run_bass_kernel_spmd(nc, in_maps, core_ids=list(range(NCORES)))

    v = np.stack([r["V"] for r in res.results])  # (8, 2, 768)
    v_text = v[:4].reshape(B, H)
    v_img = v[4:].reshape(B, H)
    att_text = np.broadcast_to(v_text[:, None, :], (B, S, H)).copy()
    att_img = np.broadcast_to(v_img[:, None, :], (B, S, H)).copy()
    return att_text, att_img
